# revision 39
# baseline (speedup 1.0000x reference)
"""Trainium2 Bass kernel for DualGraphConvolution — v4 (pipelined, fused DVE).

reference math (N=8192, D=512):
    node_att = softmax(x @ node_w, axis=0)            # [N, 1]
    h        = x @ edge_w                             # [N, D]
    e        = h @ h.T ; masked where adj <= 0        # [N, N]
    edge_att = softmax(e, axis=1)                     # [N, N]
    out      = (adj * node_att * edge_att) @ (x @ weight) + bias

Key identities (same as v3):
    e = (x @ M) @ x.T            with M = edge_w @ edge_w.T  (512x512)
    (T @ (x @ W)) = (T @ x) @ W  (T = masked/scaled attention weights)
Row-shard over 8 cores with rotation; no collectives.

v4 changes over v3:
  - mask-add + row-max fused into one DVE tensor_tensor_reduce per sweep
    (was scalar_tensor_tensor + tensor_reduce = 2 passes)
  - e_sb stored fp16 (halves SBUF + read traffic; |e|<=~60, ULP 0.03)
  - initial x DMAs ordered by first use across 4 queues (SP/Act/DVE/Pool)
    so block 0 never waits ~20us for xr tiles
  - software pipelining: PE order is T(b,h) then e(b+1,h), so the softmax
    tail (DVE mask/max -> Act exp) of one half hides under the next
    block's e-matmuls
  - ttps->SBUF copies moved to the (otherwise idle) Pool engine
"""

import numpy as np

import concourse.bass as bass
import concourse.mybir as mybir
import concourse.tile as tile
from concourse import bacc
from concourse.bass_utils import run_bass_kernel_spmd

F8 = mybir.dt.float8e4
F16 = mybir.dt.float16
F32 = mybir.dt.float32
ALU = mybir.AluOpType
ACTF = mybir.ActivationFunctionType
AX = mybir.AxisListType

N = 8192
D = 512
NCORES = 8
JS = 1024            # columns per sweep
HALFS = 2
MASK_NEG = -1000.0
NEG_INIT = -3.0e38


def build_program(n=N, d=D, ncores=NCORES, js=JS):
    loc = n // ncores          # 1024 rows owned by this core
    rb = loc // 128            # 8 row blocks per core
    kc = d // 128              # 4 contraction chunks
    nsweep = n // js           # 8
    sph = nsweep // HALFS      # 4
    jc = js // 128             # 8 128-col chunks per sweep

    nc = bacc.Bacc("TRN2", target_bir_lowering=False, debug=False,
                   num_devices=ncores)

    xt_d = nc.dram_tensor("xt", [kc, 128, n], F16, kind="ExternalInput")
    xr_d = nc.dram_tensor("xr", [n // 128, 128, d], F16, kind="ExternalInput")
    adjm_d = nc.dram_tensor("adjm", [loc, n], F8, kind="ExternalInput")
    ewt_d = nc.dram_tensor("ewt", [kc, 128, d], F16, kind="ExternalInput")
    wt_d = nc.dram_tensor("wt", [kc, 128, d], F16, kind="ExternalInput")
    nw_d = nc.dram_tensor("nw", [kc, 128, 1], F16, kind="ExternalInput")
    bias_d = nc.dram_tensor("biasb", [128, d], F16, kind="ExternalInput")
    id_d = nc.dram_tensor("ident", [128, 128], F16, kind="ExternalInput")
    out_d = nc.dram_tensor("out", [loc, d], F32, kind="ExternalOutput")

    with tile.TileContext(nc) as tc:
        with (
            tc.tile_pool(name="const", bufs=1) as constp,
            tc.tile_pool(name="big", bufs=1) as bigp,
            tc.tile_pool(name="vec", bufs=4) as vecp,
            tc.tile_pool(name="adjp", bufs=4) as adjp,
        ):
            # ---- constants + streamed x, ordered by first use ----
            # The DMA engine pool behaves ~serially at ~350GB/s, so global
            # transfer order == config order.  Alternate SP/Act queues in
            # strict first-use order; prefetch block-0 adjm between the xt
            # tiles it follows.
            # single queue (SP): one dma_start spreads over all 16 SDMA
            # engines, and FIFO order per queue makes the load order exactly
            # the first-use order.  Act/DVE/Pool sequencers stay free for
            # compute (a DMA config blocks its sequencer while the backlog
            # drains).
            def dq():
                return nc.sync

            ewt_sb = constp.tile([128, kc, d], F16)
            dq().dma_start(ewt_sb[:], ewt_d.rearrange("c p f -> p c f"))
            nw_sb = constp.tile([128, kc, 1], F16)
            dq().dma_start(nw_sb[:], nw_d.rearrange("c p f -> p c f"))
            id_sb = constp.tile([128, 128], F16)
            dq().dma_start(id_sb[:], id_d[:])

            # xt_g[g][p, c, r] = x[g*1024+r, c*128+p]
            # xr_g[g][p, u, f] = x[(g*8+u)*128+p, f]
            xt_g = [bigp.tile([128, kc, js], F16, name=f"xtg{g}")
                    for g in range(ncores)]
            xr_g = [bigp.tile([128, jc, d], F16, name=f"xrg{g}")
                    for g in range(ncores)]
            xt_view = xt_d.rearrange("c p r -> p c r")
            adj_pre = {}

            sphjs = (nsweep // HALFS) * js

            def adjm_fetch(b, h):
                # one 512KB transfer per half-phase (fewer DMA fixed costs)
                t = adjp.tile([128, sphjs], F8, tag="adj", name="adj")
                nc.sync.dma_start(
                    t[:], adjm_d[b * 128:(b + 1) * 128,
                                 h * sphjs:(h + 1) * sphjs])
                return t

            for g in range(ncores):
                dq().dma_start(xt_g[g][:],
                               xt_view[:, :, g * js:(g + 1) * js])
                # after every other xt tile, prefetch a half-phase adjm
                # slab so blocks 0-1 never wait on the mask stream
                if g % 2 == 1:
                    pb, ph = divmod(g // 2, 2)
                    adj_pre[pb, ph] = adjm_fetch(pb, ph)
            for g in range(ncores):
                dq().dma_start(
                    xr_g[g][:],
                    xr_d[g * jc:(g + 1) * jc].rearrange("rc p f -> p rc f"))
            # needed only at O(0), ~35us in
            wt_sb = constp.tile([128, kc, d], F16)
            dq().dma_start(wt_sb[:], wt_d.rearrange("c p f -> p c f"))
            bias_sb = constp.tile([128, d], F16)
            dq().dma_start(bias_sb[:], bias_d[:])

            ones_dummy = constp.tile([1, 1], F32)
            nc.vector.memset(ones_dummy[:], 0.0)
            # preload the Exp activation table during the initial DMA wait
            warm = constp.tile([1, 1], F32)
            nc.scalar.activation(warm[:], ones_dummy[:], ACTF.Exp)

            gT_loc = bigp.tile([128, kc, loc], F16)  # g[r,dd] at [dd%128, dd//128, r]
            pzp = bigp.tile([1, 16], F32)            # per-rt sums of exp(p)
            nc.vector.memset(pzp[:], 0.0)
            scale0 = bigp.tile([128, rb], F32)
            pel = bigp.tile([1, loc], F32)           # exp(p) for local rows

            # ---- phase 0: M = Ew Ew^T, g_loc = x_loc M ----
            with (
                tc.tile_pool(name="ph0", bufs=2) as ph0p,
                tc.tile_pool(name="ph0ps", bufs=2, space="PSUM") as ph0ps,
            ):
                # PE p-state warmup: run throwaway matmuls on a zeroed tile
                # while the first DMAs land, so phase 0 starts at 2.4GHz
                # instead of 0.65GHz.
                wz = ph0p.tile([128, 512], F16, tag="wz")
                nc.vector.memset(wz[:], 0.0)
                wps = ph0ps.tile([128, 512], F32, tag="mps")
                for i in range(14):
                    nc.tensor.matmul(wps[:], wz[:, 0:128], wz[:],
                                     start=(i == 0), stop=(i == 13))
                m_sb = ph0p.tile([128, kc, d], F16, tag="m")
                for ab in range(kc):
                    mps = ph0ps.tile([128, d], F32, tag="mps")
                    for fc in range(kc):
                        nc.tensor.matmul(
                            mps[:], ewt_sb[:, fc, ab * 128:(ab + 1) * 128],
                            ewt_sb[:, fc, :], start=(fc == 0),
                            stop=(fc == kc - 1))
                    nc.vector.tensor_copy(out=m_sb[:, ab, :], in_=mps[:])
                # gT_loc: stationary M chunks, moving local x^T
                for rt in range(loc // 512):
                    rsl = slice((rt % 2) * 512, (rt % 2) * 512 + 512)
                    for dc in range(kc):
                        gps = ph0ps.tile([128, 512], F32, tag="gps")
                        for c2 in range(kc):
                            nc.tensor.matmul(
                                gps[:], m_sb[:, c2, dc * 128:(dc + 1) * 128],
                                xt_g[rt // 2][:, c2, rsl],
                                start=(c2 == 0), stop=(c2 == kc - 1))
                        nc.vector.tensor_copy(
                            out=gT_loc[:, dc, rt * 512:(rt + 1) * 512],
                            in_=gps[:])

            # ---- main loop (software-pipelined) ----
            with (
                tc.tile_pool(name="ep", bufs=3) as ep,
                tc.tile_pool(name="tp", bufs=2) as tp,
                tc.tile_pool(name="ttp", bufs=2) as ttp,
                tc.tile_pool(name="otp", bufs=1) as otp,
                tc.tile_pool(name="statp", bufs=3) as statp,
                tc.tile_pool(name="pscrp", bufs=1) as pscrp,
                tc.tile_pool(name="outp", bufs=1) as outp,
                tc.tile_pool(name="epsp", bufs=2, space="PSUM") as epsp,
                tc.tile_pool(name="ttpsp", bufs=2, space="PSUM") as ttpsp,
                tc.tile_pool(name="sp", bufs=1, space="PSUM") as spp,
            ):
                st = {}   # per-(b,h) live tiles handed from ephase to tphase

                def ephase(b, h):
                    if h == 0:
                        st[b, "mstk"] = statp.tile([128, nsweep], F32,
                                                   tag="mstk", name="mstk")
                        st[b, "zstk"] = statp.tile([128, nsweep], F32,
                                                   tag="zstk", name="zstk")
                    mstk = st[b, "mstk"]
                    e_sb = ep.tile([128, sph, js], F16, tag="esb")
                    st[b, h] = e_sb
                    if (b, h) in adj_pre:
                        adj_t = adj_pre.pop((b, h))
                    else:
                        adj_t = adjm_fetch(b, h)
                    for qq in range(sph):
                        q = h * sph + qq
                        eps = epsp.tile([128, js], F32, tag="eps")
                        for c in range(kc):
                            for j2 in range(js // 512):
                                sl = slice(j2 * 512, (j2 + 1) * 512)
                                nc.tensor.matmul(
                                    eps[:, sl],
                                    gT_loc[:, c, b * 128:(b + 1) * 128],
                                    xt_g[q][:, c, sl],
                                    start=(c == 0), stop=(c == kc - 1))
                        # e_sb = eps + adjm (DVE); rowmax on the idle Pool
                        # engine (tensor_tensor_reduce hard-crashes the DVE
                        # exec unit on TRN2 hardware)
                        nc.vector.scalar_tensor_tensor(
                            out=e_sb[:, qq], in0=eps[:], scalar=1.0,
                            in1=adj_t[:, qq * js:(qq + 1) * js],
                            op0=ALU.mult, op1=ALU.add)
                        nc.vector.tensor_reduce(
                            mstk[:, q:q + 1], e_sb[:, qq], axis=AX.X,
                            op=ALU.max)
                        if b == 0:
                            # node attention p = x @ nw for this x group
                            for rr in range(2):
                                rt = q * 2 + rr
                                pps_t = epsp.tile([128, js], F32, tag="eps")
                                pps = pps_t[:]
                                for c in range(kc):
                                    nc.tensor.matmul(
                                        pps[0:1, 0:512],
                                        nw_sb[:, c, :],
                                        xt_g[q][:, c,
                                                rr * 512:(rr + 1) * 512],
                                        start=(c == 0), stop=(c == kc - 1))
                                if q == 0:
                                    pdst = pel[:, rt * 512:(rt + 1) * 512]
                                else:
                                    pscr = pscrp.tile([1, 512], F32,
                                                      tag="pescr")
                                    pdst = pscr[:]
                                nc.scalar.activation(
                                    pdst, pps[0:1, 0:512], ACTF.Exp,
                                    accum_out=pzp[:, rt:rt + 1])
                    nmx = vecp.tile([128, 1], F32, tag=f"nmx{h}")
                    nc.vector.tensor_reduce(
                        nmx[:], mstk[:, h * sph:(h + 1) * sph],
                        axis=AX.X, op=ALU.max, negate=True)
                    st[b, h, "nmx"] = nmx
                    if b == 0 and h == HALFS - 1:
                        # pz complete: scale0 = exp(p_loc)/pz via K=1 mms
                        pz = vecp.tile([1, 1], F32, tag="pz")
                        nc.vector.reduce_sum(pz[:], pzp[:], axis=AX.X)
                        pzi = vecp.tile([1, 1], F32, tag="pzi")
                        nc.vector.reciprocal(pzi[:], pz[:])
                        sps_t = epsp.tile([128, js], F32, tag="eps")
                        sps_ = sps_t[:]
                        for i in range(rb):
                            nc.tensor.matmul(
                                sps_[:, i:i + 1],
                                pel[:, i * 128:(i + 1) * 128],
                                pzi[:])
                        nc.vector.tensor_copy(out=scale0[:],
                                              in_=sps_[:, 0:rb])

                def tphase(b, h):
                    e_sb = st.pop((b, h))
                    mstk = st[b, "mstk"]
                    zstk = st[b, "zstk"]
                    nmx = st[b, h, "nmx"]
                    if h == 0:
                        st[b, "S"] = [
                            spp.tile([128, d], F32, name=f"Sh{hh}",
                                     tag=f"S{hh}")
                            for hh in range(HALFS)]
                    S = st[b, "S"]
                    for qq in range(sph):
                        q = h * sph + qq
                        t_t = tp.tile([128, js], F16, tag="t")
                        nc.scalar.activation(
                            t_t[:], e_sb[:, qq], ACTF.Exp, bias=nmx[:],
                            accum_out=zstk[:, q:q + 1])
                        ttps = ttpsp.tile([128, js], F16, tag="ttps")
                        for u in range(jc):
                            nc.tensor.transpose(
                                ttps[:, u * 128:(u + 1) * 128],
                                t_t[:, u * 128:(u + 1) * 128], id_sb[:])
                        tt_sb = ttp.tile([128, js], F16, tag="tt")
                        if qq % 2 == 0:
                            nc.vector.tensor_copy(out=tt_sb[:], in_=ttps[:])
                        else:
                            nc.scalar.copy(tt_sb[:], ttps[:])
                        for u in range(jc):
                            nc.tensor.matmul(
                                S[h][:], tt_sb[:, u * 128:(u + 1) * 128],
                                xr_g[q][:, u, :],
                                start=(qq == 0 and u == 0),
                                stop=(qq == sph - 1 and u == jc - 1))

                def opath(b):
                    mstk = st.pop((b, "mstk"))
                    zstk = st.pop((b, "zstk"))
                    S = st.pop((b, "S"))
                    nmh = [st.pop((b, h, "nmx")) for h in range(HALFS)]
                    # combine halves -> O' = c0*S0 + c1*S1 (f16)
                    nm = vecp.tile([128, 1], F32, tag="nm")
                    nc.vector.tensor_tensor(nm[:], nmh[0][:], nmh[1][:],
                                            ALU.min)
                    ch = []
                    for h in range(HALFS):
                        dfh = vecp.tile([128, 1], F32, tag=f"df{h}")
                        nc.vector.tensor_tensor(dfh[:], nm[:], nmh[h][:],
                                                ALU.subtract)
                        cfh = vecp.tile([128, 1], F32, tag=f"cf{h}")
                        nc.scalar.activation(cfh[:], dfh[:], ACTF.Exp)
                        ch.append(cfh)
                    oc_a = otp.tile([128, d], F16, tag="oca")
                    nc.vector.tensor_scalar_mul(oc_a[:], S[0][:], ch[0][:])
                    oc = otp.tile([128, d], F16, tag="oc")
                    nc.vector.scalar_tensor_tensor(
                        out=oc[:], in0=S[1][:], scalar=ch[1][:], in1=oc_a[:],
                        op0=ALU.mult, op1=ALU.add)
                    # O = (O' @ W) * (scale0 / Z) + bias
                    otps = ttpsp.tile([128, js], F16, tag="ttps")
                    for fc in range(kc):
                        nc.tensor.transpose(
                            otps[:, fc * 128:(fc + 1) * 128],
                            oc[:, fc * 128:(fc + 1) * 128], id_sb[:])
                    ot_sb = otp.tile([128, d], F16, tag="ot")
                    nc.vector.tensor_copy(out=ot_sb[:], in_=otps[:, 0:d])
                    ops_t = epsp.tile([128, d], F32, tag="eps",
                                      name="ops")
                    for fc in range(kc):
                        nc.tensor.matmul(
                            ops_t[:], ot_sb[:, fc * 128:(fc + 1) * 128],
                            wt_sb[:, fc, :], start=(fc == 0),
                            stop=(fc == kc - 1))
                    # Z and final scale
                    zh = vecp.tile([128, HALFS], F32, tag="zh")
                    nc.vector.tensor_reduce(
                        zh[:], zstk[:].rearrange("p (h q) -> p h q", h=HALFS),
                        axis=AX.X, op=ALU.add)
                    zc0 = vecp.tile([128, 1], F32, tag="zc0")
                    nc.vector.tensor_tensor(zc0[:], zh[:, 0:1], ch[0][:],
                                            ALU.mult)
                    Z = vecp.tile([128, 1], F32, tag="Z")
                    nc.vector.scalar_tensor_tensor(
                        out=Z[:], in0=zh[:, 1:2], scalar=ch[1][:], in1=zc0[:],
                        op0=ALU.mult, op1=ALU.add)
                    zi = vecp.tile([128, 1], F32, tag="zi")
                    nc.vector.reciprocal(zi[:], Z[:])
                    sc = vecp.tile([128, 1], F32, tag="sc")
                    nc.vector.tensor_tensor(sc[:], zi[:], scale0[:, b:b + 1],
                                            ALU.mult)
                    o_t = outp.tile([128, d], F32, tag="o")
                    nc.vector.scalar_tensor_tensor(
                        out=o_t[:], in0=ops_t[:], scalar=sc[:],
                        in1=bias_sb[:], op0=ALU.mult, op1=ALU.add)
                    nc.scalar.dma_start(out_d[b * 128:(b + 1) * 128, :],
                                        o_t[:])

                # depth-3 pipeline: e(0,0) e(0,1) e(1,0) | T(0,0) e(1,1)
                # T(0,1) e(2,0) O(0) | T(1,0) e(2,1) T(1,1) e(3,0) O(1) ...
                # The extra e-phase in flight keeps PE fed through the
                # DMA-bound startup and the per-half softmax tails.
                ephase(0, 0)
                ephase(0, 1)
                ephase(1, 0)
                for b in range(rb):
                    tphase(b, 0)
                    if b + 1 < rb:
                        ephase(b + 1, 1)
                    tphase(b, 1)
                    if b + 2 < rb:
                        ephase(b + 2, 0)
                    opath(b)

    nc.finalize()
    return nc


def make_in_maps(x, adj, weight, bias, node_w, edge_w, n=N, d=D, ncores=NCORES):
    loc = n // ncores
    kc = d // 128
    ewt = np.ascontiguousarray(edge_w.T.astype(np.float16)).reshape(kc, 128, d)
    wt = np.ascontiguousarray(weight.astype(np.float16)).reshape(kc, 128, d)
    nw = np.ascontiguousarray(node_w.astype(np.float16)).reshape(kc, 128, 1)
    biasb = np.ascontiguousarray(
        np.broadcast_to(bias.astype(np.float16)[None, :], (128, d)))
    ident = np.eye(128, dtype=np.float16)
    import ml_dtypes
    x16 = x.astype(np.float16)
    adj16 = adj.astype(np.float16)
    in_maps = []
    for c in range(ncores):
        sh = c * loc
        x_rot = np.roll(x16, -sh, axis=0)
        xt_c = np.ascontiguousarray(x_rot.T).reshape(kc, 128, n)
        xr_c = np.ascontiguousarray(x_rot).reshape(n // 128, 128, d)
        # mask add-in: 0 where adj>0, -240 (exact in e4m3) where masked
        adjm_c = np.ascontiguousarray(
            (np.roll(adj16[sh:sh + loc], -sh, axis=1) - 1) * 240.0
        ).astype(ml_dtypes.float8_e4m3)
        in_maps.append({"xt": xt_c, "xr": xr_c, "adjm": adjm_c, "ewt": ewt,
                        "wt": wt, "nw": nw, "biasb": biasb, "ident": ident})
    return in_maps


_CACHE = {}


def kernel(x, adj, weight, bias, node_w, edge_w):
    x = np.asarray(x)
    adj = np.asarray(adj)
    weight = np.asarray(weight)
    bias = np.asarray(bias)
    node_w = np.asarray(node_w)
    edge_w = np.asarray(edge_w)
    assert x.shape == (N, D) and adj.shape == (N, N)
    if "nc" not in _CACHE:
        _CACHE["nc"] = build_program()
    nc = _CACHE["nc"]
    in_maps = make_in_maps(x, adj, weight, bias, node_w, edge_w)
    res = run_bass_kernel_spmd(nc, in_maps, list(range(NCORES)))
    out = np.concatenate([res.results[c]["out"] for c in range(NCORES)], axis=0)
    return np.ascontiguousarray(out.astype(np.float32))


# revision 41
# speedup vs baseline: 1.1191x; 1.1191x over previous
"""Trainium2 Bass kernel for DualGraphConvolution — v4 (pipelined, fused DVE).

reference math (N=8192, D=512):
    node_att = softmax(x @ node_w, axis=0)            # [N, 1]
    h        = x @ edge_w                             # [N, D]
    e        = h @ h.T ; masked where adj <= 0        # [N, N]
    edge_att = softmax(e, axis=1)                     # [N, N]
    out      = (adj * node_att * edge_att) @ (x @ weight) + bias

Key identities (same as v3):
    e = (x @ M) @ x.T            with M = edge_w @ edge_w.T  (512x512)
    (T @ (x @ W)) = (T @ x) @ W  (T = masked/scaled attention weights)
Row-shard over 8 cores with rotation; no collectives.

v4 changes over v3 (sim 357us -> 316us, HW chain 1.43ms -> 1.14ms):
  - depth-3 software pipeline: PE order is ... T(b,0) e(b+1,1) T(b,1)
    e(b+2,0) O(b) ..., so softmax tails (DVE mask/max -> Act exp) and the
    DMA-bound startup hide under e-matmuls of later blocks; ep bufs=3
  - ALL input DMAs on the single SP queue in exact first-use order
    (ewt, xt tiles interleaved with block-0/1 adjm prefetch, xr, wt/bias).
    One dma_start spreads over all 16 SDMA engines, FIFO per queue, and
    keeping Act/DVE/Pool free of DMA configs matters: a config blocks its
    sequencer while the DMA backlog drains.
  - adjm stored fp8e4 (mask add-in is 0/-240, both exact; halves traffic)
  - e_sb stored fp16 (halves SBUF + read traffic; |e|<=~60, ULP 0.03)
  - PE p-state warmup matmuls before phase 0 (PE ramps 0.65->2.4GHz)
  - phase-0 PSUM->SBUF copies on DVE (Act's queue must stay empty early)
  - NOT USED: DVE tensor_tensor_reduce (fuses mask+max) hard-crashes the
    DVE exec unit on TRN2 hardware (NRT_EXEC_UNIT_UNRECOVERABLE);
    GPSIMD/Pool cannot access PSUM, and Pool tensor_reduce is
    partition-axis (C) only.
"""

import numpy as np

import concourse.bass as bass
import concourse.mybir as mybir
import concourse.tile as tile
from concourse import bacc
from concourse.bass_utils import run_bass_kernel_spmd

F8 = mybir.dt.float8e4
F16 = mybir.dt.float16
F32 = mybir.dt.float32
ALU = mybir.AluOpType
ACTF = mybir.ActivationFunctionType
AX = mybir.AxisListType

N = 8192
D = 512
NCORES = 8
JS = 1024            # columns per sweep
HALFS = 2
MASK_NEG = -1000.0
NEG_INIT = -3.0e38


def build_program(n=N, d=D, ncores=NCORES, js=JS):
    loc = n // ncores          # 1024 rows owned by this core
    rb = loc // 128            # 8 row blocks per core
    kc = d // 128              # 4 contraction chunks
    nsweep = n // js           # 8
    sph = nsweep // HALFS      # 4
    jc = js // 128             # 8 128-col chunks per sweep

    nc = bacc.Bacc("TRN2", target_bir_lowering=False, debug=False,
                   num_devices=ncores)

    xt_d = nc.dram_tensor("xt", [kc, 128, n], F16, kind="ExternalInput")
    xr_d = nc.dram_tensor("xr", [n // 128, 128, d], F16, kind="ExternalInput")
    adjm_d = nc.dram_tensor("adjm", [loc, n], F8, kind="ExternalInput")
    ewt_d = nc.dram_tensor("ewt", [kc, 128, d], F16, kind="ExternalInput")
    wt_d = nc.dram_tensor("wt", [kc, 128, d], F16, kind="ExternalInput")
    nw_d = nc.dram_tensor("nw", [kc, 128, 1], F16, kind="ExternalInput")
    bias_d = nc.dram_tensor("biasb", [128, d], F32, kind="ExternalInput")
    id_d = nc.dram_tensor("ident", [128, 128], F16, kind="ExternalInput")
    out_d = nc.dram_tensor("out", [loc, d], F32, kind="ExternalOutput")

    with tile.TileContext(nc) as tc:
        with (
            tc.tile_pool(name="const", bufs=1) as constp,
            tc.tile_pool(name="big", bufs=1) as bigp,
            tc.tile_pool(name="vec", bufs=2) as vecp,
            tc.tile_pool(name="adjp", bufs=12) as adjp,
        ):
            # ---- constants + streamed x, ordered by first use ----
            # The DMA engine pool behaves ~serially at ~350GB/s, so global
            # transfer order == config order.  Alternate SP/Act queues in
            # strict first-use order; prefetch block-0 adjm between the xt
            # tiles it follows.
            # single queue (SP): one dma_start spreads over all 16 SDMA
            # engines, and FIFO order per queue makes the load order exactly
            # the first-use order.  Act/DVE/Pool sequencers stay free for
            # compute (a DMA config blocks its sequencer while the backlog
            # drains).
            def dq():
                return nc.sync

            ewt_sb = constp.tile([128, kc, d], F16)
            dq().dma_start(ewt_sb[:], ewt_d.rearrange("c p f -> p c f"))
            nw_sb = constp.tile([128, kc, 1], F16)
            dq().dma_start(nw_sb[:], nw_d.rearrange("c p f -> p c f"))
            id_sb = constp.tile([128, 128], F16)
            dq().dma_start(id_sb[:], id_d[:])

            # xt_g[g][p, c, r] = x[g*1024+r, c*128+p]
            # xr_g[g][p, u, f] = x[(g*8+u)*128+p, f]
            xt_g = [bigp.tile([128, kc, js], F16, name=f"xtg{g}")
                    for g in range(ncores)]
            xr_g = [bigp.tile([128, jc, d], F16, name=f"xrg{g}")
                    for g in range(ncores)]
            xt_view = xt_d.rearrange("c p r -> p c r")
            adj_pre = {}

            def adjm_fetch(b, h, qq):
                q = h * (nsweep // HALFS) + qq
                t = adjp.tile([128, js], F8, tag="adj", name="adj")
                nc.sync.dma_start(
                    t[:], adjm_d[b * 128:(b + 1) * 128,
                                 q * js:(q + 1) * js])
                return t

            for g in range(ncores):
                dq().dma_start(xt_g[g][:],
                               xt_view[:, :, g * js:(g + 1) * js])
                # after each xt tile, prefetch two adjm tiles so blocks 0-1
                # never wait on the mask stream
                for k in range(2):
                    idx = g * 2 + k          # 0..15 -> (b,h,qq)
                    pb, rem = divmod(idx, 8)
                    ph, pq = divmod(rem, 4)
                    adj_pre[pb, ph, pq] = adjm_fetch(pb, ph, pq)
            for g in range(ncores):
                dq().dma_start(
                    xr_g[g][:],
                    xr_d[g * jc:(g + 1) * jc].rearrange("rc p f -> p rc f"))
            # needed only at O(0), ~35us in
            wt_sb = constp.tile([128, kc, d], F16)
            dq().dma_start(wt_sb[:], wt_d.rearrange("c p f -> p c f"))
            bias_sb = constp.tile([128, d], F32)
            dq().dma_start(bias_sb[:], bias_d[:])

            ones_dummy = constp.tile([1, 1], F32)
            nc.vector.memset(ones_dummy[:], 0.0)
            # preload the Exp activation table during the initial DMA wait
            warm = constp.tile([1, 1], F32)
            nc.scalar.activation(warm[:], ones_dummy[:], ACTF.Exp)

            gT_loc = bigp.tile([128, kc, loc], F16)  # g[r,dd] at [dd%128, dd//128, r]
            pzp = bigp.tile([1, 16], F32)            # per-rt sums of exp(p)
            nc.vector.memset(pzp[:], 0.0)
            scale0 = bigp.tile([128, rb], F32)
            pel = bigp.tile([1, loc], F32)           # exp(p) for local rows

            # ---- phase 0: M = Ew Ew^T, g_loc = x_loc M ----
            with (
                tc.tile_pool(name="ph0", bufs=2) as ph0p,
                tc.tile_pool(name="ph0ps", bufs=2, space="PSUM") as ph0ps,
            ):
                # PE p-state warmup: run throwaway matmuls on a zeroed tile
                # while the first DMAs land, so phase 0 starts at 2.4GHz
                # instead of 0.65GHz.
                wz = ph0p.tile([128, 512], F16, tag="wz")
                nc.vector.memset(wz[:], 0.0)
                wps = ph0ps.tile([128, 512], F32, tag="mps")
                for i in range(14):
                    nc.tensor.matmul(wps[:], wz[:, 0:128], wz[:],
                                     start=(i == 0), stop=(i == 13))
                m_sb = ph0p.tile([128, kc, d], F16, tag="m")
                for ab in range(kc):
                    mps = ph0ps.tile([128, d], F32, tag="mps")
                    for fc in range(kc):
                        nc.tensor.matmul(
                            mps[:], ewt_sb[:, fc, ab * 128:(ab + 1) * 128],
                            ewt_sb[:, fc, :], start=(fc == 0),
                            stop=(fc == kc - 1))
                    nc.vector.tensor_copy(out=m_sb[:, ab, :], in_=mps[:])
                # gT_loc: stationary M chunks, moving local x^T
                for rt in range(loc // 512):
                    rsl = slice((rt % 2) * 512, (rt % 2) * 512 + 512)
                    for dc in range(kc):
                        gps = ph0ps.tile([128, 512], F32, tag="gps")
                        for c2 in range(kc):
                            nc.tensor.matmul(
                                gps[:], m_sb[:, c2, dc * 128:(dc + 1) * 128],
                                xt_g[rt // 2][:, c2, rsl],
                                start=(c2 == 0), stop=(c2 == kc - 1))
                        nc.vector.tensor_copy(
                            out=gT_loc[:, dc, rt * 512:(rt + 1) * 512],
                            in_=gps[:])

            # ---- main loop (software-pipelined) ----
            with (
                tc.tile_pool(name="ep", bufs=3) as ep,
                tc.tile_pool(name="tp", bufs=2) as tp,
                tc.tile_pool(name="ttp", bufs=2) as ttp,
                tc.tile_pool(name="otp", bufs=2) as otp,
                tc.tile_pool(name="statp", bufs=2) as statp,
                tc.tile_pool(name="pscrp", bufs=1) as pscrp,
                tc.tile_pool(name="outp", bufs=2) as outp,
                tc.tile_pool(name="epsp", bufs=2, space="PSUM") as epsp,
                tc.tile_pool(name="ttpsp", bufs=2, space="PSUM") as ttpsp,
                tc.tile_pool(name="sp", bufs=1, space="PSUM") as spp,
            ):
                st = {}   # per-(b,h) live tiles handed from ephase to tphase

                def ephase(b, h):
                    if h == 0:
                        st[b, "mstk"] = statp.tile([128, nsweep], F32,
                                                   tag="mstk", name="mstk")
                        st[b, "zstk"] = statp.tile([128, nsweep], F32,
                                                   tag="zstk", name="zstk")
                    mstk = st[b, "mstk"]
                    e_sb = ep.tile([128, sph, js], F16, tag="esb")
                    st[b, h] = e_sb
                    for qq in range(sph):
                        q = h * sph + qq
                        if (b, h, qq) in adj_pre:
                            adj_t = adj_pre.pop((b, h, qq))
                        else:
                            adj_t = adjm_fetch(b, h, qq)
                        eps = epsp.tile([128, js], F32, tag="eps")
                        for c in range(kc):
                            for j2 in range(js // 512):
                                sl = slice(j2 * 512, (j2 + 1) * 512)
                                nc.tensor.matmul(
                                    eps[:, sl],
                                    gT_loc[:, c, b * 128:(b + 1) * 128],
                                    xt_g[q][:, c, sl],
                                    start=(c == 0), stop=(c == kc - 1))
                        # e_sb = eps + adjm (DVE); rowmax on the idle Pool
                        # engine (tensor_tensor_reduce hard-crashes the DVE
                        # exec unit on TRN2 hardware)
                        nc.vector.scalar_tensor_tensor(
                            out=e_sb[:, qq], in0=eps[:], scalar=1.0,
                            in1=adj_t[:], op0=ALU.mult, op1=ALU.add)
                        nc.vector.tensor_reduce(
                            mstk[:, q:q + 1], e_sb[:, qq], axis=AX.X,
                            op=ALU.max)
                        if b == 0:
                            # node attention p = x @ nw for this x group
                            for rr in range(2):
                                rt = q * 2 + rr
                                pps_t = epsp.tile([128, js], F32, tag="eps")
                                pps = pps_t[:]
                                for c in range(kc):
                                    nc.tensor.matmul(
                                        pps[0:1, 0:512],
                                        nw_sb[:, c, :],
                                        xt_g[q][:, c,
                                                rr * 512:(rr + 1) * 512],
                                        start=(c == 0), stop=(c == kc - 1))
                                if q == 0:
                                    pdst = pel[:, rt * 512:(rt + 1) * 512]
                                else:
                                    pscr = pscrp.tile([1, 512], F32,
                                                      tag="pescr")
                                    pdst = pscr[:]
                                nc.scalar.activation(
                                    pdst, pps[0:1, 0:512], ACTF.Exp,
                                    accum_out=pzp[:, rt:rt + 1])
                    nmx = vecp.tile([128, 1], F32, tag=f"nmx{h}")
                    nc.vector.tensor_reduce(
                        nmx[:], mstk[:, h * sph:(h + 1) * sph],
                        axis=AX.X, op=ALU.max, negate=True)
                    st[b, h, "nmx"] = nmx
                    if b == 0 and h == HALFS - 1:
                        # pz complete: scale0 = exp(p_loc)/pz via K=1 mms
                        pz = vecp.tile([1, 1], F32, tag="pz")
                        nc.vector.reduce_sum(pz[:], pzp[:], axis=AX.X)
                        pzi = vecp.tile([1, 1], F32, tag="pzi")
                        nc.vector.reciprocal(pzi[:], pz[:])
                        sps_t = epsp.tile([128, js], F32, tag="eps")
                        sps_ = sps_t[:]
                        for i in range(rb):
                            nc.tensor.matmul(
                                sps_[:, i:i + 1],
                                pel[:, i * 128:(i + 1) * 128],
                                pzi[:])
                        nc.vector.tensor_copy(out=scale0[:],
                                              in_=sps_[:, 0:rb])

                def tphase(b, h):
                    e_sb = st.pop((b, h))
                    mstk = st[b, "mstk"]
                    zstk = st[b, "zstk"]
                    nmx = st[b, h, "nmx"]
                    if h == 0:
                        st[b, "S"] = [
                            spp.tile([128, d], F32, name=f"Sh{hh}",
                                     tag=f"S{hh}")
                            for hh in range(HALFS)]
                    S = st[b, "S"]
                    for qq in range(sph):
                        q = h * sph + qq
                        t_t = tp.tile([128, js], F16, tag="t")
                        nc.scalar.activation(
                            t_t[:], e_sb[:, qq], ACTF.Exp, bias=nmx[:],
                            accum_out=zstk[:, q:q + 1])
                        ttps = ttpsp.tile([128, js], F16, tag="ttps")
                        for u in range(jc):
                            nc.tensor.transpose(
                                ttps[:, u * 128:(u + 1) * 128],
                                t_t[:, u * 128:(u + 1) * 128], id_sb[:])
                        tt_sb = ttp.tile([128, js], F16, tag="tt")
                        if qq % 2 == 0:
                            nc.vector.tensor_copy(out=tt_sb[:], in_=ttps[:])
                        else:
                            nc.scalar.copy(tt_sb[:], ttps[:])
                        for u in range(jc):
                            nc.tensor.matmul(
                                S[h][:], tt_sb[:, u * 128:(u + 1) * 128],
                                xr_g[q][:, u, :],
                                start=(qq == 0 and u == 0),
                                stop=(qq == sph - 1 and u == jc - 1))

                def opath(b):
                    mstk = st.pop((b, "mstk"))
                    zstk = st.pop((b, "zstk"))
                    S = st.pop((b, "S"))
                    nmh = [st.pop((b, h, "nmx")) for h in range(HALFS)]
                    # combine halves -> O' = c0*S0 + c1*S1 (f16)
                    nm = vecp.tile([128, 1], F32, tag="nm")
                    nc.vector.tensor_tensor(nm[:], nmh[0][:], nmh[1][:],
                                            ALU.min)
                    ch = []
                    for h in range(HALFS):
                        dfh = vecp.tile([128, 1], F32, tag=f"df{h}")
                        nc.vector.tensor_tensor(dfh[:], nm[:], nmh[h][:],
                                                ALU.subtract)
                        cfh = vecp.tile([128, 1], F32, tag=f"cf{h}")
                        nc.scalar.activation(cfh[:], dfh[:], ACTF.Exp)
                        ch.append(cfh)
                    oc_a = otp.tile([128, d], F16, tag="oca")
                    nc.vector.tensor_scalar_mul(oc_a[:], S[0][:], ch[0][:])
                    oc = otp.tile([128, d], F16, tag="oc")
                    nc.vector.scalar_tensor_tensor(
                        out=oc[:], in0=S[1][:], scalar=ch[1][:], in1=oc_a[:],
                        op0=ALU.mult, op1=ALU.add)
                    # O = (O' @ W) * (scale0 / Z) + bias
                    otps = ttpsp.tile([128, js], F16, tag="ttps")
                    for fc in range(kc):
                        nc.tensor.transpose(
                            otps[:, fc * 128:(fc + 1) * 128],
                            oc[:, fc * 128:(fc + 1) * 128], id_sb[:])
                    ot_sb = otp.tile([128, d], F16, tag="ot")
                    nc.vector.tensor_copy(out=ot_sb[:], in_=otps[:, 0:d])
                    ops_t = spp.tile([128, d], F32, tag="S0")
                    for fc in range(kc):
                        nc.tensor.matmul(
                            ops_t[:], ot_sb[:, fc * 128:(fc + 1) * 128],
                            wt_sb[:, fc, :], start=(fc == 0),
                            stop=(fc == kc - 1))
                    # Z and final scale
                    zh = vecp.tile([128, HALFS], F32, tag="zh")
                    nc.vector.tensor_reduce(
                        zh[:], zstk[:].rearrange("p (h q) -> p h q", h=HALFS),
                        axis=AX.X, op=ALU.add)
                    zc0 = vecp.tile([128, 1], F32, tag="zc0")
                    nc.vector.tensor_tensor(zc0[:], zh[:, 0:1], ch[0][:],
                                            ALU.mult)
                    Z = vecp.tile([128, 1], F32, tag="Z")
                    nc.vector.scalar_tensor_tensor(
                        out=Z[:], in0=zh[:, 1:2], scalar=ch[1][:], in1=zc0[:],
                        op0=ALU.mult, op1=ALU.add)
                    zi = vecp.tile([128, 1], F32, tag="zi")
                    nc.vector.reciprocal(zi[:], Z[:])
                    sc = vecp.tile([128, 1], F32, tag="sc")
                    nc.vector.tensor_tensor(sc[:], zi[:], scale0[:, b:b + 1],
                                            ALU.mult)
                    o_t = outp.tile([128, d], F32, tag="o")
                    nc.vector.scalar_tensor_tensor(
                        out=o_t[:], in0=ops_t[:], scalar=sc[:],
                        in1=bias_sb[:], op0=ALU.mult, op1=ALU.add)
                    nc.scalar.dma_start(out_d[b * 128:(b + 1) * 128, :],
                                        o_t[:])

                # depth-3 pipeline: e(0,0) e(0,1) e(1,0) | T(0,0) e(1,1)
                # T(0,1) e(2,0) O(0) | T(1,0) e(2,1) T(1,1) e(3,0) O(1) ...
                # The extra e-phase in flight keeps PE fed through the
                # DMA-bound startup and the per-half softmax tails.
                ephase(0, 0)
                ephase(0, 1)
                ephase(1, 0)
                for b in range(rb):
                    tphase(b, 0)
                    if b + 1 < rb:
                        ephase(b + 1, 1)
                    tphase(b, 1)
                    if b + 2 < rb:
                        ephase(b + 2, 0)
                    opath(b)

    nc.finalize()
    return nc


def make_in_maps(x, adj, weight, bias, node_w, edge_w, n=N, d=D, ncores=NCORES):
    loc = n // ncores
    kc = d // 128
    ewt = np.ascontiguousarray(edge_w.T.astype(np.float16)).reshape(kc, 128, d)
    wt = np.ascontiguousarray(weight.astype(np.float16)).reshape(kc, 128, d)
    nw = np.ascontiguousarray(node_w.astype(np.float16)).reshape(kc, 128, 1)
    biasb = np.ascontiguousarray(
        np.broadcast_to(bias.astype(np.float32)[None, :], (128, d)))
    ident = np.eye(128, dtype=np.float16)
    import ml_dtypes
    x16 = x.astype(np.float16)
    adj16 = adj.astype(np.float16)
    in_maps = []
    for c in range(ncores):
        sh = c * loc
        x_rot = np.roll(x16, -sh, axis=0)
        xt_c = np.ascontiguousarray(x_rot.T).reshape(kc, 128, n)
        xr_c = np.ascontiguousarray(x_rot).reshape(n // 128, 128, d)
        # mask add-in: 0 where adj>0, -240 (exact in e4m3) where masked
        adjm_c = np.ascontiguousarray(
            (np.roll(adj16[sh:sh + loc], -sh, axis=1) - 1) * 240.0
        ).astype(ml_dtypes.float8_e4m3)
        in_maps.append({"xt": xt_c, "xr": xr_c, "adjm": adjm_c, "ewt": ewt,
                        "wt": wt, "nw": nw, "biasb": biasb, "ident": ident})
    return in_maps


_CACHE = {}


def kernel(x, adj, weight, bias, node_w, edge_w):
    x = np.asarray(x)
    adj = np.asarray(adj)
    weight = np.asarray(weight)
    bias = np.asarray(bias)
    node_w = np.asarray(node_w)
    edge_w = np.asarray(edge_w)
    assert x.shape == (N, D) and adj.shape == (N, N)
    if "nc" not in _CACHE:
        _CACHE["nc"] = build_program()
    nc = _CACHE["nc"]
    in_maps = make_in_maps(x, adj, weight, bias, node_w, edge_w)
    res = run_bass_kernel_spmd(nc, in_maps, list(range(NCORES)))
    out = np.concatenate([res.results[c]["out"] for c in range(NCORES)], axis=0)
    return np.ascontiguousarray(out.astype(np.float32))
